# revision 5
# baseline (speedup 1.0000x reference)
"""Trainium2 Bass kernel for nn_CensoredLoss_Sub.

reference:
    out = outputs.reshape(B, T, D)                     # D = 2
    loss1 = targets[:, :, 0:1] * log((1 - out) + eps)
    loss2 = targets[:, :, 1:2] * log(out + eps)
    loss  = sum((loss1 + loss2) * weights[:, :, None], axis=(0, 1))  # (D,)
    return -loss / (B * T)

Strategy (pure data-parallel over B across 8 cores; per-core partials
gathered and reduced on host — the (D,)=2-float all-reduce is trivial):

  Key identity: for both d=0,1 the coefficient of log(1-o_d+eps) is w*t0
  and the coefficient of log(o_d+eps) is w*t1. So per tile:
    ACT:  L1p = log(1-o_d+eps) for parity p (even=d0 / odd=d1), bf16,
          contiguous; same for L2p = log(o_d+eps).       (4 ACTIVATEs)
    Pool: X = w*t_even (=w*t0), Y = w*t_odd (=w*t1), bf16. (2 tensor_mul)
    DVE:  acc += sum(X*L1e), sum(Y*L2e)  -> d0 partials
          acc += sum(X*L1o), sum(Y*L2o)  -> d1 partials   (4 bf16 STTs, 2x mode)
  Host sums per-core accumulator tiles and applies -1/(B*T).
"""

import numpy as np

B, T, D = 16384, 512, 2
N_CORES = 8
EPS = 1e-8
P = 128

# per-core flattened sizes
FO = (B // N_CORES) * T * D // P  # o/t columns per partition = 16384
F_TILE = 4096                     # o-elems per partition per tile
N_ITER = FO // F_TILE

PREP_ENGINE = "gpsimd"            # which engine computes X/Y = w*t

_compiled = {}


def _build():
    import concourse.mybir as mybir
    from concourse import bacc
    from concourse.tile import TileContext

    f32 = mybir.dt.float32
    bf16 = mybir.dt.bfloat16
    Ln = mybir.ActivationFunctionType.Ln
    mult = mybir.AluOpType.mult

    nc = bacc.Bacc(
        "TRN2",
        target_bir_lowering=False,
        debug=False,
        num_devices=N_CORES,
    )
    o_d = nc.dram_tensor("o", [P, FO], f32, kind="ExternalInput").ap()
    t_d = nc.dram_tensor("t", [P, FO], f32, kind="ExternalInput").ap()
    w_d = nc.dram_tensor("w", [P, FO // 2], f32, kind="ExternalInput").ap()
    acc_d = nc.dram_tensor("acc", [P, 4 * N_ITER], f32, kind="ExternalOutput").ap()

    FP = F_TILE // 2  # pairs per partition per tile

    with TileContext(nc) as tc:
        with (
            tc.tile_pool(name="io", bufs=3) as io_pool,
            tc.tile_pool(name="mid", bufs=2) as mid_pool,
            tc.tile_pool(name="accp", bufs=1) as acc_pool,
        ):
            acc = acc_pool.tile([P, 4 * N_ITER], f32)
            bias_eps = acc_pool.tile([P, 1], f32)
            bias_1eps = acc_pool.tile([P, 1], f32)
            nc.vector.memset(bias_eps[:], EPS)
            nc.vector.memset(bias_1eps[:], 1.0 + EPS)
            prep = getattr(nc, PREP_ENGINE)
            for i in range(N_ITER):
                o = io_pool.tile([P, FP, 2], f32, tag="o")
                t = io_pool.tile([P, FP, 2], f32, tag="t")
                w = io_pool.tile([P, FP], f32, tag="w")
                sl = slice(i * F_TILE, (i + 1) * F_TILE)
                nc.sync.dma_start(out=o[:].rearrange("p f d -> p (f d)"), in_=o_d[:, sl])
                nc.sync.dma_start(out=t[:].rearrange("p f d -> p (f d)"), in_=t_d[:, sl])
                nc.sync.dma_start(out=w[:], in_=w_d[:, i * FP : (i + 1) * FP])

                # logs, deinterleaved by parity d, contiguous bf16 outputs
                l1e = mid_pool.tile([P, FP], bf16, tag="l1e")
                l1o = mid_pool.tile([P, FP], bf16, tag="l1o")
                l2e = mid_pool.tile([P, FP], bf16, tag="l2e")
                l2o = mid_pool.tile([P, FP], bf16, tag="l2o")
                nc.scalar.activation(l1e[:], o[:, :, 0], Ln, bias=bias_1eps[:], scale=-1.0)
                nc.scalar.activation(l2e[:], o[:, :, 0], Ln, bias=bias_eps[:], scale=1.0)
                nc.scalar.activation(l1o[:], o[:, :, 1], Ln, bias=bias_1eps[:], scale=-1.0)
                nc.scalar.activation(l2o[:], o[:, :, 1], Ln, bias=bias_eps[:], scale=1.0)

                # X = w*t0, Y = w*t1 (bf16 out)
                x = mid_pool.tile([P, FP], bf16, tag="x")
                y = mid_pool.tile([P, FP], bf16, tag="y")
                prep.tensor_mul(x[:], w[:], t[:, :, 0])
                prep.tensor_mul(y[:], w[:], t[:, :, 1])

                # acc columns: 4i+0: X*L1e (d0), 4i+1: Y*L2e (d0),
                #              4i+2: X*L1o (d1), 4i+3: Y*L2o (d1)
                scr = mid_pool.tile([P, FP], bf16, tag="scr")
                for k, (a, b) in enumerate(((x, l1e), (y, l2e), (x, l1o), (y, l2o))):
                    nc.vector.scalar_tensor_tensor(
                        out=scr[:], in0=a[:], scalar=1.0, in1=b[:],
                        op0=mult, op1=mult,
                        accum_out=acc[:, 4 * i + k : 4 * i + k + 1],
                    )
            nc.sync.dma_start(out=acc_d, in_=acc[:])
    nc.compile()
    return nc


def _get_nc():
    if "nc" not in _compiled:
        _compiled["nc"] = _build()
    return _compiled["nc"]


def make_in_maps(outputs, targets, weights):
    rows = B // N_CORES
    in_maps = []
    for c in range(N_CORES):
        sh = slice(c * rows, (c + 1) * rows)
        in_maps.append(
            {
                "o": np.ascontiguousarray(outputs[sh]).reshape(P, FO),
                "t": np.ascontiguousarray(targets[sh]).reshape(P, FO),
                "w": np.ascontiguousarray(weights[sh]).reshape(P, FO // 2),
            }
        )
    return in_maps


def run_raw(in_maps, **kw):
    from concourse import bass_utils

    nc = _get_nc()
    return bass_utils.run_bass_kernel_spmd(
        nc, in_maps, core_ids=list(range(N_CORES)), **kw
    )


def finish(results) -> np.ndarray:
    total = np.zeros(2, dtype=np.float64)
    for r in results:
        a = r["acc"].astype(np.float64).reshape(P, -1, 4)
        total[0] += a[:, :, 0].sum() + a[:, :, 1].sum()
        total[1] += a[:, :, 2].sum() + a[:, :, 3].sum()
    return (-total / (B * T)).astype(np.float32)


def kernel(outputs: np.ndarray, targets: np.ndarray, weights: np.ndarray) -> np.ndarray:
    res = run_raw(make_in_maps(outputs, targets, weights))
    return finish(res.results)


# revision 12
# speedup vs baseline: 1.2793x; 1.2793x over previous
"""Trainium2 Bass kernel for nn_CensoredLoss_Sub.

reference:
    out = outputs.reshape(B, T, D)                     # D = 2
    loss1 = targets[:, :, 0:1] * log((1 - out) + eps)
    loss2 = targets[:, :, 1:2] * log(out + eps)
    loss  = sum((loss1 + loss2) * weights[:, :, None], axis=(0, 1))  # (D,)
    return -loss / (B * T)

Strategy: pure data-parallel over B across 8 cores; per-core partial sums
are gathered and reduced on host (the (D,)=2-float all-reduce is trivial).

Key identity: for both d=0,1 the coefficient of log(1-o_d+eps) is w*t0 and
the coefficient of log(o_d+eps) is w*t1:
    loss_d = sum_pairs  (w*t0)*log(1-o_d+eps) + (w*t1)*log(o_d+eps)

Host-side layout (pure permutation, no arithmetic): per tile, o is
deinterleaved into [o0|o1] (f32 — it must stay f32: bf16 rounds o to
exactly 1.0 for ~0.2% of elements and 1-o+eps then underflows to 0 ->
Ln(0) = -inf, a catastrophic-cancellation path), and t/w are packed into
one [t0|t1|w] block that a single SWDGE DMA casts f32->bf16 in the DMA
datapath. Everything on-chip reads/writes contiguously (strided APs break
DVE 2x packing; ACT scattered writes run ~5x slow).

Per tile (F o-elems, FP=F/2 pairs per partition):
  ACT:  l1 = [log(1-o0+eps)|log(1-o1+eps)], l2 = [log(o0+eps)|log(o1+eps)]
        (4 ACTIVATEs, Ln, scale/bias fused, bf16)
  DVE:  X = w*t0, Y = w*t1; P1 = X_bcast*l1, P2 = Y_bcast*l2 (all 2x bf16)
  PE:   ones[128,1]^T @ P-chunks accumulated into psum_d0/psum_d1 [1,512]
        (partition-sum; column association is irrelevant — all summed later)
Final: ACT copies psum banks to SBUF, DMA [1,1024] out; host sums and
applies -1/(B*T).
"""

import numpy as np

B, T, D = 16384, 512, 2
N_CORES = 8
EPS = 1e-8
P = 128

FO = (B // N_CORES) * T * D // P  # o/t columns per partition = 16384
F_TILE = 4096                     # o-elems per partition per tile
N_ITER = FO // F_TILE
FP = F_TILE // 2                  # pairs per partition per tile
FB = 3 * FP                       # packed t/w block columns per tile
MM_N = 512                        # matmul moving free dim

_compiled = {}


def _build():
    import concourse.mybir as mybir
    from concourse import bacc
    from concourse.tile import TileContext

    f32 = mybir.dt.float32
    bf16 = mybir.dt.bfloat16
    Ln = mybir.ActivationFunctionType.Ln
    Copy = mybir.ActivationFunctionType.Copy

    nc = bacc.Bacc(
        "TRN2",
        target_bir_lowering=False,
        debug=False,
        num_devices=N_CORES,
    )
    o_d = nc.dram_tensor("o", [P, FO], f32, kind="ExternalInput").ap()
    tw_d = nc.dram_tensor("tw", [P, N_ITER * FB], f32, kind="ExternalInput").ap()
    acc_d = nc.dram_tensor("acc", [1, 2 * MM_N], f32, kind="ExternalOutput").ap()

    with TileContext(nc) as tc:
        with (
            tc.tile_pool(name="io", bufs=3) as io_pool,
            tc.tile_pool(name="mid", bufs=2) as mid_pool,
            tc.tile_pool(name="one", bufs=1) as one_pool,
            tc.tile_pool(name="ps", bufs=1, space="PSUM") as psum_pool,
        ):
            bias_eps = one_pool.tile([P, 1], f32)
            bias_1eps = one_pool.tile([P, 1], f32)
            ones = one_pool.tile([P, 1], bf16)
            res = one_pool.tile([1, 2 * MM_N], f32)
            nc.vector.memset(bias_eps[:], EPS)
            nc.vector.memset(bias_1eps[:], 1.0 + EPS)
            nc.vector.memset(ones[:], 1.0)
            psum0 = psum_pool.tile([1, MM_N], f32, tag="ps0")
            psum1 = psum_pool.tile([1, MM_N], f32, tag="ps1")
            psum = [psum0, psum1]

            for i in range(N_ITER):
                ot = io_pool.tile([P, 2, FP], f32, tag="ot")
                blk = io_pool.tile([P, FB], bf16, tag="blk")
                nc.sync.dma_start(
                    out=ot[:].rearrange("p d f -> p (d f)"),
                    in_=o_d[:, i * F_TILE : (i + 1) * F_TILE],
                )
                nc.gpsimd.dma_start(out=blk[:], in_=tw_d[:, i * FB : (i + 1) * FB])
                o = ot[:]                                      # [P, 2, FP] f32
                tw = blk[:].rearrange("p (c f) -> p c f", c=3)  # [P, 3, FP] bf16

                l1 = mid_pool.tile([P, 2, FP], bf16, tag="l1")
                l2 = mid_pool.tile([P, 2, FP], bf16, tag="l2")
                for dd in range(2):
                    nc.scalar.activation(l1[:, dd, :], o[:, dd, :], Ln, bias=bias_1eps[:], scale=-1.0)
                    nc.scalar.activation(l2[:, dd, :], o[:, dd, :], Ln, bias=bias_eps[:], scale=1.0)

                x = mid_pool.tile([P, FP], bf16, tag="x")
                y = mid_pool.tile([P, FP], bf16, tag="y")
                nc.vector.tensor_mul(x[:], tw[:, 2, :], tw[:, 0, :])
                nc.vector.tensor_mul(y[:], tw[:, 2, :], tw[:, 1, :])

                p1 = mid_pool.tile([P, 2, FP], bf16, tag="p1")
                p2 = mid_pool.tile([P, 2, FP], bf16, tag="p2")
                nc.vector.tensor_mul(p1[:], x[:].unsqueeze(1).broadcast_to([P, 2, FP]), l1[:])
                nc.vector.tensor_mul(p2[:], y[:].unsqueeze(1).broadcast_to([P, 2, FP]), l2[:])

                # partition-sum every product column into psum_d[parity]
                for pi, prod in enumerate((p1, p2)):
                    for dd in range(2):
                        for c in range(FP // MM_N):
                            first = i == 0 and pi == 0 and c == 0
                            last = (
                                i == N_ITER - 1
                                and pi == 1
                                and c == FP // MM_N - 1
                            )
                            nc.tensor.matmul(
                                psum[dd][:],
                                ones[:],
                                prod[:, dd, c * MM_N : (c + 1) * MM_N],
                                start=first,
                                stop=last,
                            )

            for dd in range(2):
                nc.scalar.activation(res[:, dd * MM_N : (dd + 1) * MM_N], psum[dd][:], Copy, bias=0.0, scale=1.0)
            nc.sync.dma_start(out=acc_d, in_=res[:])
    nc.compile()
    return nc


def _get_nc():
    if "nc" not in _compiled:
        _compiled["nc"] = _build()
    return _compiled["nc"]


def _deint(x2d):
    """[P, FO] interleaved -> per-tile [d0-block | d1-block] layout."""
    return np.ascontiguousarray(
        x2d.reshape(P, N_ITER, FP, 2).transpose(0, 1, 3, 2)
    ).reshape(P, FO)


def _pack_tw(t2d, w2d):
    """Pack [P,FO] t (interleaved) + [P,FO/2] w into per-tile [t0|t1|w]
    blocks -> [P, N_ITER*3*FP]. Pure permutation/concatenation."""
    tv = t2d.reshape(P, N_ITER, FP, 2)
    wv = w2d.reshape(P, N_ITER, 1, FP)
    blk = np.concatenate([tv.transpose(0, 1, 3, 2), wv], axis=2)
    return np.ascontiguousarray(blk).reshape(P, N_ITER * FB)


def make_in_maps(outputs, targets, weights):
    rows = B // N_CORES
    in_maps = []
    for c in range(N_CORES):
        sh = slice(c * rows, (c + 1) * rows)
        in_maps.append(
            {
                "o": _deint(np.ascontiguousarray(outputs[sh]).reshape(P, FO)),
                "tw": _pack_tw(
                    np.ascontiguousarray(targets[sh]).reshape(P, FO),
                    np.ascontiguousarray(weights[sh]).reshape(P, FO // 2),
                ),
            }
        )
    return in_maps


def run_raw(in_maps, **kw):
    from concourse import bass_utils

    nc = _get_nc()
    return bass_utils.run_bass_kernel_spmd(
        nc, in_maps, core_ids=list(range(N_CORES)), **kw
    )


def finish(results) -> np.ndarray:
    total = np.zeros(2, dtype=np.float64)
    for r in results:
        a = r["acc"].astype(np.float64).reshape(2, MM_N)
        total[0] += a[0].sum()
        total[1] += a[1].sum()
    return (-total / (B * T)).astype(np.float32)


def kernel(outputs: np.ndarray, targets: np.ndarray, weights: np.ndarray) -> np.ndarray:
    res = run_raw(make_in_maps(outputs, targets, weights))
    return finish(res.results)


# revision 14
# speedup vs baseline: 1.2890x; 1.0076x over previous
"""Trainium2 Bass kernel for nn_CensoredLoss_Sub.

reference:
    out = outputs.reshape(B, T, D)                     # D = 2
    loss1 = targets[:, :, 0:1] * log((1 - out) + eps)
    loss2 = targets[:, :, 1:2] * log(out + eps)
    loss  = sum((loss1 + loss2) * weights[:, :, None], axis=(0, 1))  # (D,)
    return -loss / (B * T)

Strategy: pure data-parallel over B across 8 cores; per-core partial sums
are gathered and reduced on host (the (D,)=2-float all-reduce is trivial).

Key identity: for both d=0,1 the coefficient of log(1-o_d+eps) is w*t0 and
the coefficient of log(o_d+eps) is w*t1:
    loss_d = sum_pairs  (w*t0)*log(1-o_d+eps) + (w*t1)*log(o_d+eps)

Host-side layout (pure permutation, no arithmetic): per tile, o is
deinterleaved into [o0|o1] (f32 — it must stay f32: bf16 rounds o to
exactly 1.0 for ~0.2% of elements and 1-o+eps then underflows to 0 ->
Ln(0) = -inf, a catastrophic-cancellation path), and t/w are packed into
one [t0|t1|w] block that a single SWDGE DMA casts f32->bf16 in the DMA
datapath. Everything on-chip reads/writes contiguously (strided APs break
DVE 2x packing; ACT scattered writes run ~5x slow).

Per tile (F o-elems, FP=F/2 pairs per partition):
  ACT:  l1 = [log(1-o0+eps)|log(1-o1+eps)], l2 = [log(o0+eps)|log(o1+eps)]
        (4 ACTIVATEs, Ln, scale/bias fused, bf16)
  DVE:  X = w*t0, Y = w*t1; P1 = X_bcast*l1, P2 = Y_bcast*l2 (all 2x bf16)
  PE:   ones[128,1]^T @ P-chunks accumulated into psum_d0/psum_d1 [1,512]
        (partition-sum; column association is irrelevant — all summed later)
Final: ACT copies psum banks to SBUF, DMA [1,1024] out; host sums and
applies -1/(B*T).
"""

import numpy as np

B, T, D = 16384, 512, 2
N_CORES = 8
EPS = 1e-8
P = 128

FO = (B // N_CORES) * T * D // P  # o/t columns per partition = 16384
# variable tile sizes: small head tile (first ACT starts early) and small
# tail tile (short last dependency chain after the final DMA byte lands)
TILES = [1024, 2048, 4096, 4096, 4096, 1024]
assert sum(TILES) == FO
MM_N = 512                        # matmul moving free dim

_compiled = {}


def _build():
    import concourse.mybir as mybir
    from concourse import bacc
    from concourse.tile import TileContext

    f32 = mybir.dt.float32
    bf16 = mybir.dt.bfloat16
    Ln = mybir.ActivationFunctionType.Ln
    Copy = mybir.ActivationFunctionType.Copy

    nc = bacc.Bacc(
        "TRN2",
        target_bir_lowering=False,
        debug=False,
        num_devices=N_CORES,
    )
    o_d = nc.dram_tensor("o", [P, FO], f32, kind="ExternalInput").ap()
    tw_d = nc.dram_tensor("tw", [P, FO + FO // 2], f32, kind="ExternalInput").ap()
    acc_d = nc.dram_tensor("acc", [1, 2 * MM_N], f32, kind="ExternalOutput").ap()

    with TileContext(nc) as tc:
        with (
            tc.tile_pool(name="io", bufs=3) as io_pool,
            tc.tile_pool(name="mid", bufs=2) as mid_pool,
            tc.tile_pool(name="one", bufs=1) as one_pool,
            tc.tile_pool(name="ps", bufs=1, space="PSUM") as psum_pool,
        ):
            bias_eps = one_pool.tile([P, 1], f32)
            bias_1eps = one_pool.tile([P, 1], f32)
            ones = one_pool.tile([P, 1], bf16)
            res = one_pool.tile([1, 2 * MM_N], f32)
            nc.vector.memset(bias_eps[:], EPS)
            nc.vector.memset(bias_1eps[:], 1.0 + EPS)
            nc.vector.memset(ones[:], 1.0)
            psum0 = psum_pool.tile([1, MM_N], f32, tag="ps0")
            psum1 = psum_pool.tile([1, MM_N], f32, tag="ps1")
            psum = [psum0, psum1]
            dummy = one_pool.tile([P, 1], bf16)
            # warm the Ln table set while the first DMA is in flight
            nc.scalar.activation(dummy[:], bias_eps[:], Ln, bias=bias_1eps[:], scale=1.0)

            FPM = max(TILES) // 2
            o_off = 0
            tw_off = 0
            for i, F in enumerate(TILES):
                FP = F // 2
                FB = 3 * FP
                ot = io_pool.tile([P, 2, FPM], f32, tag="ot")
                blk = io_pool.tile([P, 3 * FPM], bf16, tag="blk")
                nc.sync.dma_start(
                    out=ot[:, :, :FP],
                    in_=o_d[:, o_off : o_off + F].rearrange("p (d f) -> p d f", d=2),
                )
                nc.gpsimd.dma_start(out=blk[:, :FB], in_=tw_d[:, tw_off : tw_off + FB])
                o_off += F
                tw_off += FB
                o = ot[:, :, :FP]                                   # [P, 2, FP] f32
                tw = blk[:, :FB].rearrange("p (c f) -> p c f", c=3)  # [P, 3, FP] bf16

                l1 = mid_pool.tile([P, 2, FPM], bf16, tag="l1")
                l2 = mid_pool.tile([P, 2, FPM], bf16, tag="l2")
                for dd in range(2):
                    nc.scalar.activation(l1[:, dd, :FP], o[:, dd, :], Ln, bias=bias_1eps[:], scale=-1.0)
                for dd in range(2):
                    nc.scalar.activation(l2[:, dd, :FP], o[:, dd, :], Ln, bias=bias_eps[:], scale=1.0)

                x = mid_pool.tile([P, FPM], bf16, tag="x")
                y = mid_pool.tile([P, FPM], bf16, tag="y")
                nc.vector.tensor_mul(x[:, :FP], tw[:, 2, :], tw[:, 0, :])
                nc.vector.tensor_mul(y[:, :FP], tw[:, 2, :], tw[:, 1, :])

                p1 = mid_pool.tile([P, 2, FPM], bf16, tag="p1")
                p2 = mid_pool.tile([P, 2, FPM], bf16, tag="p2")
                nc.vector.tensor_mul(p1[:, :, :FP], x[:, :FP].unsqueeze(1).broadcast_to([P, 2, FP]), l1[:, :, :FP])
                nc.vector.tensor_mul(p2[:, :, :FP], y[:, :FP].unsqueeze(1).broadcast_to([P, 2, FP]), l2[:, :, :FP])

                # partition-sum every product column into psum_d[parity]
                for pi, prod in enumerate((p1, p2)):
                    for dd in range(2):
                        for c in range(FP // MM_N):
                            first = i == 0 and pi == 0 and c == 0
                            last = (
                                i == len(TILES) - 1
                                and pi == 1
                                and c == FP // MM_N - 1
                            )
                            nc.tensor.matmul(
                                psum[dd][:],
                                ones[:],
                                prod[:, dd, c * MM_N : (c + 1) * MM_N],
                                start=first,
                                stop=last,
                            )

            nc.scalar.activation(res[:, 0:MM_N], psum[0][:], Copy, bias=0.0, scale=1.0)
            nc.vector.tensor_copy(res[:, MM_N : 2 * MM_N], psum[1][:])
            nc.sync.dma_start(out=acc_d, in_=res[:])
    nc.compile()
    return nc


def _get_nc():
    if "nc" not in _compiled:
        _compiled["nc"] = _build()
    return _compiled["nc"]


def _deint(x2d):
    """[P, FO] interleaved -> per-tile [d0-block | d1-block] layout."""
    out = np.empty_like(x2d)
    off = 0
    for F in TILES:
        FP = F // 2
        blk = x2d[:, off : off + F].reshape(P, FP, 2).transpose(0, 2, 1)
        out[:, off : off + F] = blk.reshape(P, F)
        off += F
    return out


def _pack_tw(t2d, w2d):
    """Pack [P,FO] t (interleaved) + [P,FO/2] w into per-tile [t0|t1|w]
    blocks -> [P, FO + FO//2]. Pure permutation/concatenation."""
    out = np.empty((P, FO + FO // 2), dtype=t2d.dtype)
    t_off = 0
    w_off = 0
    b_off = 0
    for F in TILES:
        FP = F // 2
        tv = t2d[:, t_off : t_off + F].reshape(P, FP, 2).transpose(0, 2, 1)
        out[:, b_off : b_off + F] = tv.reshape(P, F)
        out[:, b_off + F : b_off + F + FP] = w2d[:, w_off : w_off + FP]
        t_off += F
        w_off += FP
        b_off += F + FP
    return out


def make_in_maps(outputs, targets, weights):
    rows = B // N_CORES
    in_maps = []
    for c in range(N_CORES):
        sh = slice(c * rows, (c + 1) * rows)
        in_maps.append(
            {
                "o": _deint(np.ascontiguousarray(outputs[sh]).reshape(P, FO)),
                "tw": _pack_tw(
                    np.ascontiguousarray(targets[sh]).reshape(P, FO),
                    np.ascontiguousarray(weights[sh]).reshape(P, FO // 2),
                ),
            }
        )
    return in_maps


def run_raw(in_maps, **kw):
    from concourse import bass_utils

    nc = _get_nc()
    return bass_utils.run_bass_kernel_spmd(
        nc, in_maps, core_ids=list(range(N_CORES)), **kw
    )


def finish(results) -> np.ndarray:
    total = np.zeros(2, dtype=np.float64)
    for r in results:
        a = r["acc"].astype(np.float64).reshape(2, MM_N)
        total[0] += a[0].sum()
        total[1] += a[1].sum()
    return (-total / (B * T)).astype(np.float32)


def kernel(outputs: np.ndarray, targets: np.ndarray, weights: np.ndarray) -> np.ndarray:
    res = run_raw(make_in_maps(outputs, targets, weights))
    return finish(res.results)


# revision 16
# speedup vs baseline: 1.3407x; 1.0401x over previous
"""Trainium2 Bass kernel for nn_CensoredLoss_Sub.

reference:
    out = outputs.reshape(B, T, D)                     # D = 2
    loss1 = targets[:, :, 0:1] * log((1 - out) + eps)
    loss2 = targets[:, :, 1:2] * log(out + eps)
    loss  = sum((loss1 + loss2) * weights[:, :, None], axis=(0, 1))  # (D,)
    return -loss / (B * T)

Strategy: pure data-parallel over B across 8 cores; per-core partial sums
are gathered and reduced on host (the (D,)=2-float all-reduce is trivial).

Key identity: for both d=0,1 the coefficient of log(1-o_d+eps) is w*t0 and
the coefficient of log(o_d+eps) is w*t1:
    loss_d = sum_pairs  (w*t0)*log(1-o_d+eps) + (w*t1)*log(o_d+eps)

Host-side layout (pure permutation, no arithmetic): per tile, o is
deinterleaved into [o0|o1] (f32 — it must stay f32: bf16 rounds o to
exactly 1.0 for ~0.2% of elements and 1-o+eps then underflows to 0 ->
Ln(0) = -inf, a catastrophic-cancellation path), and t/w are packed into
one [t0|t1|w] block that a single SWDGE DMA casts f32->bf16 in the DMA
datapath. Everything on-chip reads/writes contiguously (strided APs break
DVE 2x packing; ACT scattered writes run ~5x slow).

Per tile (F o-elems, FP=F/2 pairs per partition):
  ACT:  l1 = [log(1-o0+eps)|log(1-o1+eps)], l2 = [log(o0+eps)|log(o1+eps)]
        (4 ACTIVATEs, Ln, scale/bias fused, bf16)
  DVE:  X = w*t0, Y = w*t1; P1 = X_bcast*l1, P2 = Y_bcast*l2 (all 2x bf16)
  PE:   ones[128,1]^T @ P-chunks accumulated into psum_d0/psum_d1 [1,512]
        (partition-sum; column association is irrelevant — all summed later)
Final: ACT copies psum banks to SBUF, DMA [1,1024] out; host sums and
applies -1/(B*T).
"""

import numpy as np

B, T, D = 16384, 512, 2
N_CORES = 8
EPS = 1e-8
P = 128

FO = (B // N_CORES) * T * D // P  # o/t columns per partition = 16384
# variable tile sizes: small head tile (first ACT starts early) and small
# tail tile (short last dependency chain after the final DMA byte lands)
TILES = [1024, 2048, 4096, 4096, 2048, 1024, 1024, 1024]
assert sum(TILES) == FO
MM_N = 512                        # matmul moving free dim

_compiled = {}


def _build():
    import concourse.mybir as mybir
    from concourse import bacc
    from concourse.tile import TileContext

    f32 = mybir.dt.float32
    bf16 = mybir.dt.bfloat16
    Ln = mybir.ActivationFunctionType.Ln
    Copy = mybir.ActivationFunctionType.Copy

    nc = bacc.Bacc(
        "TRN2",
        target_bir_lowering=False,
        debug=False,
        num_devices=N_CORES,
    )
    o_d = nc.dram_tensor("o", [P, FO], f32, kind="ExternalInput").ap()
    tw_d = nc.dram_tensor("tw", [P, FO + FO // 2], f32, kind="ExternalInput").ap()
    acc_d = nc.dram_tensor("acc", [1, 2 * MM_N], f32, kind="ExternalOutput").ap()

    with TileContext(nc) as tc:
        with (
            tc.tile_pool(name="io", bufs=3) as io_pool,
            tc.tile_pool(name="mid", bufs=2) as mid_pool,
            tc.tile_pool(name="one", bufs=1) as one_pool,
            tc.tile_pool(name="ps", bufs=1, space="PSUM") as psum_pool,
        ):
            bias_eps = one_pool.tile([P, 1], f32)
            bias_1eps = one_pool.tile([P, 1], f32)
            ones = one_pool.tile([P, 1], bf16)
            res = one_pool.tile([1, 2 * MM_N], f32)
            nc.vector.memset(bias_eps[:], EPS)
            nc.vector.memset(bias_1eps[:], 1.0 + EPS)
            nc.vector.memset(ones[:], 1.0)
            psum0 = psum_pool.tile([1, MM_N], f32, tag="ps0")
            psum1 = psum_pool.tile([1, MM_N], f32, tag="ps1")
            psum = [psum0, psum1]
            dummy = one_pool.tile([P, 1], bf16)
            # warm the Ln table set while the first DMA is in flight
            nc.scalar.activation(dummy[:], bias_eps[:], Ln, bias=bias_1eps[:], scale=1.0)

            FPM = max(TILES) // 2
            o_off = 0
            tw_off = 0
            for i, F in enumerate(TILES):
                FP = F // 2
                FB = 3 * FP
                ot = io_pool.tile([P, 2, FPM], f32, tag="ot")
                blk = io_pool.tile([P, 3 * FPM], bf16, tag="blk")
                nc.sync.dma_start(
                    out=ot[:, :, :FP],
                    in_=o_d[:, o_off : o_off + F].rearrange("p (d f) -> p d f", d=2),
                )
                nc.gpsimd.dma_start(out=blk[:, :FB], in_=tw_d[:, tw_off : tw_off + FB])
                o_off += F
                tw_off += FB
                o = ot[:, :, :FP]                                   # [P, 2, FP] f32
                tw = blk[:, :FB].rearrange("p (c f) -> p c f", c=3)  # [P, 3, FP] bf16

                l1 = mid_pool.tile([P, 2, FPM], bf16, tag="l1")
                l2 = mid_pool.tile([P, 2, FPM], bf16, tag="l2")
                x = mid_pool.tile([P, FPM], bf16, tag="x")
                y = mid_pool.tile([P, FPM], bf16, tag="y")
                p1 = mid_pool.tile([P, 2, FPM], bf16, tag="p1")
                p2 = mid_pool.tile([P, 2, FPM], bf16, tag="p2")
                nc.vector.tensor_mul(x[:, :FP], tw[:, 2, :], tw[:, 0, :])
                nc.vector.tensor_mul(y[:, :FP], tw[:, 2, :], tw[:, 1, :])
                # per-parity chains so products/matmuls start after one ACT
                for dd in range(2):
                    nc.scalar.activation(l1[:, dd, :FP], o[:, dd, :], Ln, bias=bias_1eps[:], scale=-1.0)
                    nc.scalar.activation(l2[:, dd, :FP], o[:, dd, :], Ln, bias=bias_eps[:], scale=1.0)
                    nc.vector.tensor_mul(p1[:, dd, :FP], x[:, :FP], l1[:, dd, :FP])
                    nc.vector.tensor_mul(p2[:, dd, :FP], y[:, :FP], l2[:, dd, :FP])
                    for pi, prod in enumerate((p1, p2)):
                        for c in range(FP // MM_N):
                            first = i == 0 and pi == 0 and c == 0
                            last = (
                                i == len(TILES) - 1
                                and pi == 1
                                and c == FP // MM_N - 1
                            )
                            nc.tensor.matmul(
                                psum[dd][:],
                                ones[:],
                                prod[:, dd, c * MM_N : (c + 1) * MM_N],
                                start=first,
                                stop=last,
                            )

            nc.scalar.activation(res[:, 0:MM_N], psum[0][:], Copy, bias=0.0, scale=1.0)
            nc.vector.tensor_copy(res[:, MM_N : 2 * MM_N], psum[1][:])
            nc.sync.dma_start(out=acc_d, in_=res[:])
    nc.compile()
    return nc


def _get_nc():
    if "nc" not in _compiled:
        _compiled["nc"] = _build()
    return _compiled["nc"]


def _deint(x2d):
    """[P, FO] interleaved -> per-tile [d0-block | d1-block] layout."""
    out = np.empty_like(x2d)
    off = 0
    for F in TILES:
        FP = F // 2
        blk = x2d[:, off : off + F].reshape(P, FP, 2).transpose(0, 2, 1)
        out[:, off : off + F] = blk.reshape(P, F)
        off += F
    return out


def _pack_tw(t2d, w2d):
    """Pack [P,FO] t (interleaved) + [P,FO/2] w into per-tile [t0|t1|w]
    blocks -> [P, FO + FO//2]. Pure permutation/concatenation."""
    out = np.empty((P, FO + FO // 2), dtype=t2d.dtype)
    t_off = 0
    w_off = 0
    b_off = 0
    for F in TILES:
        FP = F // 2
        tv = t2d[:, t_off : t_off + F].reshape(P, FP, 2).transpose(0, 2, 1)
        out[:, b_off : b_off + F] = tv.reshape(P, F)
        out[:, b_off + F : b_off + F + FP] = w2d[:, w_off : w_off + FP]
        t_off += F
        w_off += FP
        b_off += F + FP
    return out


def make_in_maps(outputs, targets, weights):
    rows = B // N_CORES
    in_maps = []
    for c in range(N_CORES):
        sh = slice(c * rows, (c + 1) * rows)
        in_maps.append(
            {
                "o": _deint(np.ascontiguousarray(outputs[sh]).reshape(P, FO)),
                "tw": _pack_tw(
                    np.ascontiguousarray(targets[sh]).reshape(P, FO),
                    np.ascontiguousarray(weights[sh]).reshape(P, FO // 2),
                ),
            }
        )
    return in_maps


def run_raw(in_maps, **kw):
    from concourse import bass_utils

    nc = _get_nc()
    return bass_utils.run_bass_kernel_spmd(
        nc, in_maps, core_ids=list(range(N_CORES)), **kw
    )


def finish(results) -> np.ndarray:
    total = np.zeros(2, dtype=np.float64)
    for r in results:
        a = r["acc"].astype(np.float64).reshape(2, MM_N)
        total[0] += a[0].sum()
        total[1] += a[1].sum()
    return (-total / (B * T)).astype(np.float32)


def kernel(outputs: np.ndarray, targets: np.ndarray, weights: np.ndarray) -> np.ndarray:
    res = run_raw(make_in_maps(outputs, targets, weights))
    return finish(res.results)
